# revision 41
# baseline (speedup 1.0000x reference)
"""MultiHeadAttention Trainium2 kernel (8 NeuronCores), bf16 data path.

Sharding: 8 cores = 4 batches x 2 head-groups (8 heads each).
Core g: batch b = g//2, head-group hg = g%2 (heads hg*8 .. hg*8+7).

Device program (identical on all cores, SPMD):
  inputs (per core): xq/xk/xv = x[b].T  [1024, 2048] (bf16),
    wq/wk/wv = w[:, hg*512:(hg+1)*512]  [1024, 512] (bf16),
    wo = w_o[hg*512:(hg+1)*512, :]      [512, 1024] (bf16),
    bq = b_q slice reshaped [4, 128] (f32)
  output: yt [1024, 2048] f32 = (partial out).T for this batch/head-group,
    unnormalized by biases (host adds b_v @ w_o + b_o once per batch).

Math identities used (exact in real arithmetic):
  softmax((Q+bq)(K+bk)^T) == softmax((Q+bq) K^T)   [k-constant terms cancel]
  attn @ (V + bv) @ Wo + bo == attn @ V @ Wo + (bv @ Wo + bo)  [rows sum to 1]
  exp without max-subtraction is safe: scores peak ~8.7 << bf16 range.

Layouts: QhT/KhT [128 = head-pair d, 2048 seq] per pair; Vh [128 k-chunk,
8 heads x (64 dv + ones-col)]; ones-col makes the AV matmul also produce
Z = sum_k exp(s) at psum row 64.

Scheduling: single interleaved stream. Projections (K/Q/V) are emitted
lazily as "weave" units between attention stream items, so the exp stream
on the scalar engine starts ~6us in and the PE stays dense throughout
(HAM clock gate stays at 8/8). PSUM: proj 2 banks + scores 4 + AV 2.
"""
import numpy as np

B, S, D = 4, 2048, 1024
HPC, PAIRS, QB, KC, CC = 8, 4, 4, 16, 8  # heads/core, pairs, 512-q-blocks, 128-k-chunks, 128-c-chunks
N = 512

_CACHE = {}


def _build():
    from concourse import bacc
    import concourse.mybir as mybir
    import concourse.tile as tile

    F32 = mybir.dt.float32
    BF16 = mybir.dt.bfloat16
    AF = mybir.ActivationFunctionType

    nc = bacc.Bacc()
    xq_d = nc.declare_dram_parameter("xq", [D, S], BF16, isOutput=False)
    xk_d = nc.declare_dram_parameter("xk", [D, S], BF16, isOutput=False)
    xv_d = nc.declare_dram_parameter("xv", [D, S], BF16, isOutput=False)
    # weights host-preshuffled so loads are contiguous; wq/wk pair-major
    # [128, p, c, 128] so pair-0 can land first, wv chunk-major [128, c, n]
    wq_d = nc.declare_dram_parameter("wq", [128, CC * N], BF16, isOutput=False)
    wk_d = nc.declare_dram_parameter("wk", [128, CC * N], BF16, isOutput=False)
    wv_d = nc.declare_dram_parameter("wv", [128, CC * N], BF16, isOutput=False)
    wo_d = nc.declare_dram_parameter("wo", [128, PAIRS * D], BF16, isOutput=False)
    bq_d = nc.declare_dram_parameter("bq", [PAIRS, 128], F32, isOutput=False)
    ones_d = nc.declare_dram_parameter("ones", [128, HPC], BF16, isOutput=False)
    yt_d = nc.dram_tensor("yt", [D, S], F32, kind="ExternalOutput")

    with tile.TileContext(nc) as tc:
        with (
            tc.tile_pool(name="per", bufs=1) as per,
            tc.tile_pool(name="wp", bufs=1) as wp,
            tc.tile_pool(name="xs", bufs=1) as xsp,
            tc.tile_pool(name="ep", bufs=1) as epp,
            tc.tile_pool(name="msc", bufs=1) as msc,
            tc.tile_pool(name="pp", bufs=1, space="PSUM") as pp,
            tc.tile_pool(name="sc", bufs=1, space="PSUM") as scp,
            tc.tile_pool(name="avp", bufs=1, space="PSUM") as avp,
        ):
            # ---- persistent tiles ----
            kh = [per.tile([128, S], BF16, name=f"kh{p}", tag="kh", bufs=PAIRS)
                  for p in range(PAIRS)]
            qh = [per.tile([128, S], BF16, name=f"qh{p}", tag="qh", bufs=PAIRS)
                  for p in range(PAIRS)]
            vs = [per.tile([128, HPC * 65], BF16, name=f"vs{t}", tag="vs", bufs=KC)
                  for t in range(KC)]
            bqt = per.tile([128, PAIRS], F32, name="bqt", tag="bqt", bufs=1)
            for p in range(PAIRS):
                nc.scalar.dma_start(out=bqt[:, p:p + 1], in_=bq_d[p, :])

            # ---- weights: all four live through the interleaved stream;
            # contiguous loads spread over four queues so the first matmul
            # isn't gated on a serial weight-load chain ----
            wk_s = wp.tile([128, PAIRS, CC, 128], BF16, name="wk_s", tag="w2", bufs=4)
            wq_s = wp.tile([128, PAIRS, CC, 128], BF16, name="wq_s", tag="w2", bufs=4)
            wv_s = wp.tile([128, CC, N], BF16, name="wv_s", tag="w2", bufs=4)
            wo_s = wp.tile([128, PAIRS, D], BF16, name="wo_s", tag="w2", bufs=4)
            nc.sync.dma_start(
                out=wk_s, in_=wk_d.rearrange("p (q c n) -> p q c n",
                                             q=PAIRS, c=CC))
            nc.scalar.dma_start(
                out=wq_s, in_=wq_d.rearrange("p (q c n) -> p q c n",
                                             q=PAIRS, c=CC))
            nc.gpsimd.dma_start(out=wv_s, in_=wv_d.rearrange("p (c n) -> p c n", n=N))
            nc.scalar.dma_start(out=wo_s, in_=wo_d.rearrange("p (i n) -> p i n", n=D))

            # DMA queue alternation for x-tile loads. The first two blocks
            # (xk0/xq0, before any exp is queued) may also use the scalar
            # queue; after that scalar is reserved for the ACT exp stream.
            dma_tgl = [0]

            def xdma(out, in_):
                i = dma_tgl[0]
                dma_tgl[0] += 1
                if i < 16:
                    eng = (nc.sync, nc.gpsimd, nc.scalar)[i % 3]
                else:
                    eng = nc.sync if i % 2 == 0 else nc.gpsimd
                eng.dma_start(out=out, in_=in_)

            # ---- projection weave units (lazy, per-pair for K/Q) ----
            XB = 24  # xs slots: several j-blocks' x-tiles alive concurrently

            def load_xblock(x_d, jb, nm):
                xt = [xsp.tile([128, N], BF16, name=f"{nm}{c}", tag="xs",
                               bufs=XB) for c in range(CC)]
                for c in range(CC):
                    xdma(xt[c], x_d[128 * c:128 * (c + 1), N * jb:N * (jb + 1)])
                return xt

            def emit_v_block(q4):
                xt = load_xblock(xv_d, q4, "xvt")
                for t2 in range(4):
                    t = 4 * q4 + t2
                    ps = pp.tile([128, N], F32, name="psv", tag="proj", bufs=2)
                    for c in range(CC):
                        nc.tensor.matmul(ps, xt[c][:, 128 * t2:128 * (t2 + 1)],
                                         wv_s[:, c, :], start=(c == 0),
                                         stop=(c == CC - 1))
                    v3 = vs[t].rearrange("p (h e) -> p h e", e=65)
                    nc.sync.dma_start(out=v3[:, :, 64:65], in_=ones_d[:, :])
                    nc.vector.tensor_copy(
                        v3[:, :, 0:64], ps.rearrange("p (h e) -> p h e", e=64))

            vdone = [False] * QB
            kxt, qxt = {}, {}
            kdone, qdone = set(), set()

            def need_v(q4):
                if not vdone[q4]:
                    vdone[q4] = True
                    emit_v_block(q4)

            def _emit_k(jk, p):
                kdone.add((jk, p))
                if jk not in kxt:
                    kxt[jk] = load_xblock(xk_d, jk, "xkt")
                ps = pp.tile([128, N], F32, name="psk", tag="proj", bufs=2)
                for c in range(CC):
                    nc.tensor.matmul(ps, wk_s[:, p, c, :],
                                     kxt[jk][c], start=(c == 0), stop=(c == CC - 1))
                nc.vector.tensor_copy(kh[p][:, N * jk:N * (jk + 1)], ps)

            def need_k(jk, p=None):
                # block-emit all remaining pairs (keeps x-tile liveness
                # contiguous in emission order -> no slot-reuse deadlock)
                for p2 in range(PAIRS):
                    if (jk, p2) not in kdone:
                        _emit_k(jk, p2)

            def need_q(jq, p):
                if (jq, p) in qdone:
                    return
                qdone.add((jq, p))
                if jq not in qxt:
                    qxt[jq] = load_xblock(xq_d, jq, "xqt")
                ps = pp.tile([128, N], F32, name="psq", tag="proj", bufs=2)
                for c in range(CC):
                    nc.tensor.matmul(ps, wq_s[:, p, c, :],
                                     qxt[jq][c], start=(c == 0), stop=(c == CC - 1))
                nc.vector.tensor_scalar_add(
                    qh[p][:, N * jq:N * (jq + 1)], ps, bqt[:, p:p + 1])

            # ================= interleaved attention stream =================
            NG = KC // 2
            stream = [(j, p, g) for j in range(QB) for p in range(PAIRS)
                      for g in range(NG)]
            ctx = {}     # (j, p) -> dict(po0, po1, eA[g], eB[g])
            ots = {}     # j -> [ot tiles]
            oproj_pending = []

            def emit_scores_exp(j, p, g):
                if g == 0:
                    ctx[(j, p)] = {
                        "po0": avp.tile([65, N], F32, name="po0", tag="av", bufs=2),
                        "po1": avp.tile([65, N], F32, name="po1", tag="av", bufs=2),
                        "eA": [None] * NG, "eB": [None] * NG,
                    }
                st_ = ctx[(j, p)]
                sA = scp.tile([128, 2 * N], F32, name="sA", tag="sc", bufs=2)
                sB = scp.tile([128, 2 * N], F32, name="sB", tag="sc", bufs=2)
                for ci in range(2):
                    c = 2 * g + ci
                    nc.tensor.matmul(
                        sA[:, N * ci:N * (ci + 1)],
                        kh[p][0:64, 128 * c:128 * (c + 1)],
                        qh[p][0:64, N * j:N * (j + 1)],
                        start=True, stop=True, tile_position=(0, 0))
                    nc.tensor.matmul(
                        sB[:, N * ci:N * (ci + 1)],
                        kh[p][64:128, 128 * c:128 * (c + 1)],
                        qh[p][64:128, N * j:N * (j + 1)],
                        start=True, stop=True, tile_position=(64, 0))
                eA = epp.tile([128, 2 * N], BF16, name="eA", tag="ep", bufs=6)
                eB = epp.tile([128, 2 * N], BF16, name="eB", tag="ep", bufs=6)
                nc.scalar.activation(eA, sA, AF.Exp, scale=0.125)
                nc.scalar.activation(eB, sB, AF.Exp, scale=0.125)
                st_["eA"][g], st_["eB"][g] = eA, eB

            def emit_av(j, p, g):
                st_ = ctx[(j, p)]
                h0, h1 = 2 * p, 2 * p + 1
                for ci in range(2):
                    c = 2 * g + ci
                    ss, se = (c == 0), (c == KC - 1)
                    eAg = st_["eA"][g][:, N * ci:N * (ci + 1)]
                    eBg = st_["eB"][g][:, N * ci:N * (ci + 1)]
                    nc.tensor.matmul(
                        st_["po0"], vs[c][:, 65 * h0:65 * h0 + 65],
                        eAg, start=ss, stop=se)
                    nc.tensor.matmul(
                        st_["po1"], vs[c][:, 65 * h1:65 * h1 + 65],
                        eBg, start=ss, stop=se)

            def emit_norm(j, p):
                st_ = ctx.pop((j, p))
                if j not in ots:
                    ots[j] = [epp.tile([128, N], BF16, name=f"ot{q}", tag="ot",
                                       bufs=8) for q in range(PAIRS)]
                ot = ots[j]
                po0, po1 = st_["po0"], st_["po1"]
                # psum -> sbuf; Z rows land at raw[64, :] (halves per head)
                raw = msc.tile([65, 2 * N], F32, name="raw", tag="raw", bufs=2)
                nc.vector.tensor_copy(raw[:, 0:N], po0)
                nc.vector.tensor_copy(raw[:, N:2 * N], po1)
                zst = msc.tile([1, 2 * N], F32, name="zst", tag="zst", bufs=2)
                nc.gpsimd.dma_start(out=zst, in_=raw[64:65, :])
                zbc = msc.tile([64, 2 * N], F32, name="zbc", tag="zbc", bufs=2)
                nc.gpsimd.partition_broadcast(zbc, zst[0:1, :])
                rbc = msc.tile([64, 2 * N], F32, name="rbc", tag="rbc", bufs=2)
                nc.vector.reciprocal_approx_fast(rbc, zbc)
                nc.vector.tensor_mul(ot[p][0:64, :], raw[0:64, 0:N], rbc[:, 0:N])
                tmp1 = msc.tile([64, N], BF16, name="tmp1", tag="tmp1", bufs=2)
                nc.vector.tensor_mul(tmp1, raw[0:64, N:2 * N], rbc[:, N:2 * N])
                nc.gpsimd.dma_start(out=ot[p][64:128, :], in_=tmp1)
                if p == PAIRS - 1:
                    for e in range(8):
                        oproj_pending.append((j, e))

            def emit_oproj_chunk():
                j2, e = oproj_pending.pop(0)
                ot = ots[j2]
                py = pp.tile([128, N], F32, name="py", tag="proj", bufs=2)
                for p2 in range(PAIRS):
                    nc.tensor.matmul(py, wo_s[:, p2, 128 * e:128 * (e + 1)],
                                     ot[p2], start=(p2 == 0), stop=(p2 == PAIRS - 1))
                ys = msc.tile([128, N], F32, name="ys", tag="ys", bufs=4)
                if j2 == QB - 1 and e % 2 == 0:
                    nc.scalar.copy(ys, py)  # tail: exp stream is done, ACT idle
                else:
                    nc.vector.tensor_copy(ys, py)
                oeng = nc.sync if e % 2 == 0 else nc.gpsimd
                oeng.dma_start(
                    out=yt_d[128 * e:128 * (e + 1), N * j2:N * (j2 + 1)], in_=ys)
                if e == 7:
                    del ots[j2]

            # PE warmup while the first weight/x DMAs land: ~5us of dummy
            # matmuls releases the HAM clock throttle before real work
            wrm = msc.tile([128, N], BF16, name="wrm", tag="wrm", bufs=1)
            nc.vector.memset(wrm, 0.0)
            wps = pp.tile([128, N], F32, name="wps", tag="proj", bufs=2)
            for _ in range(10):
                nc.tensor.matmul(wps, wrm[:, 0:128], wrm, start=True, stop=True)

            LAG = 2
            # prelude: just enough for the first scores
            _emit_k(0, 0)
            need_q(0, 0)
            for idx, (j, p, g) in enumerate(stream):
                need_q(j, p)
                need_k(g // 2)
                emit_scores_exp(j, p, g)
                if idx >= LAG:
                    j2, p2, g2 = stream[idx - LAG]
                    need_v(g2 // 2)
                    emit_av(j2, p2, g2)
                    if g2 == NG - 1:
                        emit_norm(j2, p2)
                if oproj_pending:
                    emit_oproj_chunk()
            for k in range(LAG):
                j2, p2, g2 = stream[len(stream) - LAG + k]
                need_v(g2 // 2)
                emit_av(j2, p2, g2)
                if g2 == NG - 1:
                    emit_norm(j2, p2)
            while oproj_pending:
                emit_oproj_chunk()

    nc.compile()
    return nc


def _get_nc():
    if "nc" not in _CACHE:
        _CACHE["nc"] = _build()
    return _CACHE["nc"]


def kernel(q, k, v, w_q, b_q, w_k, b_k, w_v, b_v, w_o, b_o):
    import ml_dtypes

    nc = _get_nc()
    from concourse.bass_utils import run_bass_kernel_spmd

    BF = ml_dtypes.bfloat16
    q = np.asarray(q, dtype=np.float32)
    k = np.asarray(k, dtype=np.float32)
    v = np.asarray(v, dtype=np.float32)
    w_q = np.asarray(w_q, dtype=np.float32)
    w_k = np.asarray(w_k, dtype=np.float32)
    w_v = np.asarray(w_v, dtype=np.float32)
    w_o = np.asarray(w_o, dtype=np.float32)
    b_q = np.asarray(b_q, dtype=np.float32)
    b_v = np.asarray(b_v, dtype=np.float32)
    b_o = np.asarray(b_o, dtype=np.float32)

    xT = {}
    for b in range(B):
        xT[("q", b)] = np.ascontiguousarray(q[b].T.astype(BF))
        xT[("k", b)] = np.ascontiguousarray(k[b].T.astype(BF))
        xT[("v", b)] = np.ascontiguousarray(v[b].T.astype(BF))
    in_maps = []
    for g in range(8):
        b, hg = g // 2, g % 2
        sl = slice(hg * 512, (hg + 1) * 512)
        def shuf(w):  # [1024, n] -> [128, 8*n]: row c*128+p -> [p, c, :]
            n = w.shape[1]
            return np.ascontiguousarray(
                w.reshape(CC, 128, n).transpose(1, 0, 2).reshape(128, CC * n)
                .astype(BF))

        def shuf_pm(w):  # [1024, 512] -> [128, (pair, c, 128)] pair-major
            return np.ascontiguousarray(
                w.reshape(CC, 128, PAIRS, 128).transpose(1, 2, 0, 3)
                .reshape(128, PAIRS * CC * 128).astype(BF))

        wo_sl = w_o[sl, :]  # [512, 1024]: row i*128+p -> [p, i, :]
        wo_sh = np.ascontiguousarray(
            wo_sl.reshape(PAIRS, 128, D).transpose(1, 0, 2)
            .reshape(128, PAIRS * D).astype(BF))
        in_maps.append({
            "xq": xT[("q", b)], "xk": xT[("k", b)], "xv": xT[("v", b)],
            "wq": shuf_pm(w_q[:, sl]),
            "wk": shuf_pm(w_k[:, sl]),
            "wv": shuf(w_v[:, sl]),
            "wo": wo_sh,
            "bq": np.ascontiguousarray(b_q[sl].reshape(PAIRS, 128)),
            "ones": np.ones((128, HPC), dtype=BF),
        })

    _CACHE["in_maps"] = in_maps
    res = run_bass_kernel_spmd(nc, in_maps, list(range(8)), trace=False)
    outs = [r["yt"] for r in res.results]

    corr = b_v @ w_o + b_o  # [1024]
    y = np.empty((B, S, D), dtype=np.float32)
    for b in range(B):
        y[b] = outs[2 * b].T + outs[2 * b + 1].T + corr
    return y


# revision 45
# speedup vs baseline: 1.0009x; 1.0009x over previous
"""MultiHeadAttention Trainium2 kernel (8 NeuronCores), bf16 data path.

Sharding: 8 cores = 4 batches x 2 head-groups (8 heads each).
Core g: batch b = g//2, head-group hg = g%2 (heads hg*8 .. hg*8+7).

Device program (identical on all cores, SPMD):
  inputs (per core): xq/xk/xv = x[b].T  [1024, 2048] (bf16),
    wq/wk/wv = w[:, hg*512:(hg+1)*512]  [1024, 512] (bf16),
    wo = w_o[hg*512:(hg+1)*512, :]      [512, 1024] (bf16),
    bq = b_q slice reshaped [4, 128] (f32)
  output: yt [1024, 2048] f32 = (partial out).T for this batch/head-group,
    unnormalized by biases (host adds b_v @ w_o + b_o once per batch).

Math identities used (exact in real arithmetic):
  softmax((Q+bq)(K+bk)^T) == softmax((Q+bq) K^T)   [k-constant terms cancel]
  attn @ (V + bv) @ Wo + bo == attn @ V @ Wo + (bv @ Wo + bo)  [rows sum to 1]
  exp without max-subtraction is safe: scores peak ~8.7 << bf16 range.

Layouts: QhT/KhT [128 = head-pair d, 2048 seq] per pair; Vh [128 k-chunk,
8 heads x (64 dv + ones-col)]; ones-col makes the AV matmul also produce
Z = sum_k exp(s) at psum row 64.

Scheduling: single interleaved stream. Projections (K/Q/V) are emitted
lazily as "weave" units between attention stream items, so the exp stream
on the scalar engine starts ~6us in and the PE stays dense throughout
(HAM clock gate stays at 8/8). PSUM: proj 2 banks + scores 4 + AV 2.
"""
import numpy as np

B, S, D = 4, 2048, 1024
HPC, PAIRS, QB, KC, CC = 8, 4, 4, 16, 8  # heads/core, pairs, 512-q-blocks, 128-k-chunks, 128-c-chunks
N = 512

_CACHE = {}


def _build():
    from concourse import bacc
    import concourse.mybir as mybir
    import concourse.tile as tile

    F32 = mybir.dt.float32
    BF16 = mybir.dt.bfloat16
    AF = mybir.ActivationFunctionType

    nc = bacc.Bacc()
    xq_d = nc.declare_dram_parameter("xq", [D, S], BF16, isOutput=False)
    xk_d = nc.declare_dram_parameter("xk", [D, S], BF16, isOutput=False)
    xv_d = nc.declare_dram_parameter("xv", [D, S], BF16, isOutput=False)
    # weights host-preshuffled so loads are contiguous; wq/wk pair-major
    # [128, p, c, 128] so pair-0 can land first, wv chunk-major [128, c, n]
    wq_d = nc.declare_dram_parameter("wq", [128, CC * N], BF16, isOutput=False)
    wk_d = nc.declare_dram_parameter("wk", [128, CC * N], BF16, isOutput=False)
    wv_d = nc.declare_dram_parameter("wv", [128, CC * N], BF16, isOutput=False)
    wo_d = nc.declare_dram_parameter("wo", [128, PAIRS * D], BF16, isOutput=False)
    bq_d = nc.declare_dram_parameter("bq", [PAIRS, 128], F32, isOutput=False)
    ones_d = nc.declare_dram_parameter("ones", [128, HPC], BF16, isOutput=False)
    yt_d = nc.dram_tensor("yt", [D, S], F32, kind="ExternalOutput")

    with tile.TileContext(nc) as tc:
        with (
            tc.tile_pool(name="per", bufs=1) as per,
            tc.tile_pool(name="wp", bufs=1) as wp,
            tc.tile_pool(name="xs", bufs=1) as xsp,
            tc.tile_pool(name="ep", bufs=1) as epp,
            tc.tile_pool(name="msc", bufs=1) as msc,
            tc.tile_pool(name="pp", bufs=1, space="PSUM") as pp,
            tc.tile_pool(name="sc", bufs=1, space="PSUM") as scp,
            tc.tile_pool(name="avp", bufs=1, space="PSUM") as avp,
        ):
            # ---- persistent tiles ----
            kh = [per.tile([128, S], BF16, name=f"kh{p}", tag="kh", bufs=PAIRS)
                  for p in range(PAIRS)]
            qh = [per.tile([128, S], BF16, name=f"qh{p}", tag="qh", bufs=PAIRS)
                  for p in range(PAIRS)]
            vs = [per.tile([128, HPC * 65], BF16, name=f"vs{t}", tag="vs", bufs=KC)
                  for t in range(KC)]
            bqt = per.tile([128, PAIRS], F32, name="bqt", tag="bqt", bufs=1)
            for p in range(PAIRS):
                nc.scalar.dma_start(out=bqt[:, p:p + 1], in_=bq_d[p, :])

            # ---- weights: all four live through the interleaved stream;
            # contiguous loads spread over four queues so the first matmul
            # isn't gated on a serial weight-load chain ----
            wk_s = wp.tile([128, PAIRS, CC, 128], BF16, name="wk_s", tag="w2", bufs=4)
            wq_s = wp.tile([128, PAIRS, CC, 128], BF16, name="wq_s", tag="w2", bufs=4)
            wv_s = wp.tile([128, CC, N], BF16, name="wv_s", tag="w2", bufs=4)
            wo_s = wp.tile([128, PAIRS, D], BF16, name="wo_s", tag="w2", bufs=4)
            nc.sync.dma_start(
                out=wk_s, in_=wk_d.rearrange("p (q c n) -> p q c n",
                                             q=PAIRS, c=CC))
            nc.scalar.dma_start(
                out=wq_s, in_=wq_d.rearrange("p (q c n) -> p q c n",
                                             q=PAIRS, c=CC))
            nc.gpsimd.dma_start(out=wv_s, in_=wv_d.rearrange("p (c n) -> p c n", n=N))
            nc.scalar.dma_start(out=wo_s, in_=wo_d.rearrange("p (i n) -> p i n", n=D))

            # DMA queue alternation for x-tile loads. The first two blocks
            # (xk0/xq0, before any exp is queued) may also use the scalar
            # queue; after that scalar is reserved for the ACT exp stream.
            dma_tgl = [0]

            def xdma(out, in_):
                i = dma_tgl[0]
                dma_tgl[0] += 1
                if i < 16:
                    eng = (nc.sync, nc.gpsimd, nc.scalar)[i % 3]
                else:
                    eng = nc.sync if i % 2 == 0 else nc.gpsimd
                eng.dma_start(out=out, in_=in_)

            # ---- projection weave units (lazy, per-pair for K/Q) ----
            XB = 24  # xs slots: several j-blocks' x-tiles alive concurrently

            def load_xblock(x_d, jb, nm):
                xt = [xsp.tile([128, N], BF16, name=f"{nm}{c}", tag="xs",
                               bufs=XB) for c in range(CC)]
                for c in range(CC):
                    xdma(xt[c], x_d[128 * c:128 * (c + 1), N * jb:N * (jb + 1)])
                return xt

            def emit_v_block(q4):
                xt = load_xblock(xv_d, q4, "xvt")
                for t2 in range(4):
                    t = 4 * q4 + t2
                    ps = pp.tile([128, N], F32, name="psv", tag="proj", bufs=2)
                    for c in range(CC):
                        nc.tensor.matmul(ps, xt[c][:, 128 * t2:128 * (t2 + 1)],
                                         wv_s[:, c, :], start=(c == 0),
                                         stop=(c == CC - 1))
                    v3 = vs[t].rearrange("p (h e) -> p h e", e=65)
                    nc.sync.dma_start(out=v3[:, :, 64:65], in_=ones_d[:, :])
                    nc.vector.tensor_copy(
                        v3[:, :, 0:64], ps.rearrange("p (h e) -> p h e", e=64))

            vdone = [False] * QB
            kxt, qxt = {}, {}
            kdone, qdone = set(), set()

            def need_v(q4):
                if not vdone[q4]:
                    vdone[q4] = True
                    emit_v_block(q4)

            def _emit_k(jk, p):
                kdone.add((jk, p))
                if jk not in kxt:
                    kxt[jk] = load_xblock(xk_d, jk, "xkt")
                ps = pp.tile([128, N], F32, name="psk", tag="proj", bufs=2)
                for c in range(CC):
                    nc.tensor.matmul(ps, wk_s[:, p, c, :],
                                     kxt[jk][c], start=(c == 0), stop=(c == CC - 1))
                nc.vector.tensor_copy(kh[p][:, N * jk:N * (jk + 1)], ps)

            def need_k(jk, p=None):
                # block-emit all remaining pairs (keeps x-tile liveness
                # contiguous in emission order -> no slot-reuse deadlock)
                for p2 in range(PAIRS):
                    if (jk, p2) not in kdone:
                        _emit_k(jk, p2)

            def need_q(jq, p):
                if (jq, p) in qdone:
                    return
                qdone.add((jq, p))
                if jq not in qxt:
                    qxt[jq] = load_xblock(xq_d, jq, "xqt")
                ps = pp.tile([128, N], F32, name="psq", tag="proj", bufs=2)
                for c in range(CC):
                    nc.tensor.matmul(ps, wq_s[:, p, c, :],
                                     qxt[jq][c], start=(c == 0), stop=(c == CC - 1))
                nc.vector.tensor_scalar_add(
                    qh[p][:, N * jq:N * (jq + 1)], ps, bqt[:, p:p + 1])

            # ================= interleaved attention stream =================
            NG = KC // 2
            stream = [(j, p, g) for j in range(QB) for p in range(PAIRS)
                      for g in range(NG)]
            ctx = {}     # (j, p) -> dict(po0, po1, eA[g], eB[g])
            ots = {}     # j -> [ot tiles]
            oproj_pending = []

            def emit_scores_exp(j, p, g):
                if g == 0:
                    ctx[(j, p)] = {
                        "po0": avp.tile([65, N], F32, name="po0", tag="av", bufs=2),
                        "po1": avp.tile([65, N], F32, name="po1", tag="av", bufs=2),
                        "eA": [None] * NG, "eB": [None] * NG,
                    }
                st_ = ctx[(j, p)]
                sA = scp.tile([128, 2 * N], F32, name="sA", tag="sc", bufs=2)
                sB = scp.tile([128, 2 * N], F32, name="sB", tag="sc", bufs=2)
                for ci in range(2):
                    c = 2 * g + ci
                    nc.tensor.matmul(
                        sA[:, N * ci:N * (ci + 1)],
                        kh[p][0:64, 128 * c:128 * (c + 1)],
                        qh[p][0:64, N * j:N * (j + 1)],
                        start=True, stop=True, tile_position=(0, 0))
                    nc.tensor.matmul(
                        sB[:, N * ci:N * (ci + 1)],
                        kh[p][64:128, 128 * c:128 * (c + 1)],
                        qh[p][64:128, N * j:N * (j + 1)],
                        start=True, stop=True, tile_position=(64, 0))
                eA = epp.tile([128, 2 * N], BF16, name="eA", tag="ep", bufs=6)
                eB = epp.tile([128, 2 * N], BF16, name="eB", tag="ep", bufs=6)
                nc.scalar.activation(eA, sA, AF.Exp, scale=0.125)
                nc.scalar.activation(eB, sB, AF.Exp, scale=0.125)
                st_["eA"][g], st_["eB"][g] = eA, eB

            def emit_av(j, p, g):
                st_ = ctx[(j, p)]
                h0, h1 = 2 * p, 2 * p + 1
                for ci in range(2):
                    c = 2 * g + ci
                    ss, se = (c == 0), (c == KC - 1)
                    eAg = st_["eA"][g][:, N * ci:N * (ci + 1)]
                    eBg = st_["eB"][g][:, N * ci:N * (ci + 1)]
                    nc.tensor.matmul(
                        st_["po0"], vs[c][:, 65 * h0:65 * h0 + 65],
                        eAg, start=ss, stop=se)
                    nc.tensor.matmul(
                        st_["po1"], vs[c][:, 65 * h1:65 * h1 + 65],
                        eBg, start=ss, stop=se)

            def emit_norm(j, p):
                st_ = ctx.pop((j, p))
                if j not in ots:
                    ots[j] = [epp.tile([128, N], BF16, name=f"ot{q}", tag="ot",
                                       bufs=8) for q in range(PAIRS)]
                ot = ots[j]
                po0, po1 = st_["po0"], st_["po1"]
                # psum -> sbuf; Z rows land at raw[64, :] (halves per head)
                raw = msc.tile([65, 2 * N], F32, name="raw", tag="raw", bufs=2)
                nc.vector.tensor_copy(raw[:, 0:N], po0)
                nc.vector.tensor_copy(raw[:, N:2 * N], po1)
                zst = msc.tile([1, 2 * N], F32, name="zst", tag="zst", bufs=2)
                nc.gpsimd.dma_start(out=zst, in_=raw[64:65, :])
                zbc = msc.tile([64, 2 * N], F32, name="zbc", tag="zbc", bufs=2)
                nc.gpsimd.partition_broadcast(zbc, zst[0:1, :])
                rbc = msc.tile([64, 2 * N], F32, name="rbc", tag="rbc", bufs=2)
                nc.vector.reciprocal_approx_fast(rbc, zbc)
                nc.vector.tensor_mul(ot[p][0:64, :], raw[0:64, 0:N], rbc[:, 0:N])
                tmp1 = msc.tile([64, N], BF16, name="tmp1", tag="tmp1", bufs=2)
                nc.vector.tensor_mul(tmp1, raw[0:64, N:2 * N], rbc[:, N:2 * N])
                nc.gpsimd.dma_start(out=ot[p][64:128, :], in_=tmp1)
                if p == PAIRS - 1:
                    for e in range(8):
                        oproj_pending.append((j, e))

            def emit_oproj_chunk():
                j2, e = oproj_pending.pop(0)
                ot = ots[j2]
                py = pp.tile([128, N], F32, name="py", tag="proj", bufs=2)
                for p2 in range(PAIRS):
                    nc.tensor.matmul(py, wo_s[:, p2, 128 * e:128 * (e + 1)],
                                     ot[p2], start=(p2 == 0), stop=(p2 == PAIRS - 1))
                ys = msc.tile([128, N], F32, name="ys", tag="ys", bufs=4)
                if j2 == QB - 1 and e % 2 == 0:
                    nc.scalar.copy(ys, py)  # tail: exp stream is done, ACT idle
                else:
                    nc.vector.tensor_copy(ys, py)
                oeng = nc.sync if e % 2 == 0 else nc.gpsimd
                oeng.dma_start(
                    out=yt_d[128 * e:128 * (e + 1), N * j2:N * (j2 + 1)], in_=ys)
                if e == 7:
                    del ots[j2]

            # PE warmup while the first weight/x DMAs land: ~5us of dummy
            # matmuls releases the HAM clock throttle before real work
            wrm = msc.tile([128, N], BF16, name="wrm", tag="wrm", bufs=1)
            nc.vector.memset(wrm, 0.0)
            wps = pp.tile([128, N], F32, name="wps", tag="proj", bufs=2)
            for _ in range(10):
                nc.tensor.matmul(wps, wrm[:, 0:128], wrm, start=True, stop=True)

            LAG = 2
            # prelude: just enough for the first scores
            _emit_k(0, 0)
            need_q(0, 0)
            for idx, (j, p, g) in enumerate(stream):
                need_q(j, p)
                need_k(g // 2)
                emit_scores_exp(j, p, g)
                if idx >= LAG:
                    j2, p2, g2 = stream[idx - LAG]
                    need_v(g2 // 2)
                    emit_av(j2, p2, g2)
                    if g2 == NG - 1:
                        emit_norm(j2, p2)
                if oproj_pending:
                    emit_oproj_chunk()
            for k in range(LAG):
                j2, p2, g2 = stream[len(stream) - LAG + k]
                need_v(g2 // 2)
                emit_av(j2, p2, g2)
                if g2 == NG - 1:
                    emit_norm(j2, p2)
            while oproj_pending:
                emit_oproj_chunk()

    nc.compile()
    return nc


def _get_nc():
    if "nc" not in _CACHE:
        _CACHE["nc"] = _build()
    return _CACHE["nc"]


def kernel(q, k, v, w_q, b_q, w_k, b_k, w_v, b_v, w_o, b_o):
    import ml_dtypes

    nc = _get_nc()
    from concourse.bass_utils import run_bass_kernel_spmd

    BF = ml_dtypes.bfloat16
    q = np.asarray(q, dtype=np.float32)
    k = np.asarray(k, dtype=np.float32)
    v = np.asarray(v, dtype=np.float32)
    w_q = np.asarray(w_q, dtype=np.float32)
    w_k = np.asarray(w_k, dtype=np.float32)
    w_v = np.asarray(w_v, dtype=np.float32)
    w_o = np.asarray(w_o, dtype=np.float32)
    b_q = np.asarray(b_q, dtype=np.float32)
    b_v = np.asarray(b_v, dtype=np.float32)
    b_o = np.asarray(b_o, dtype=np.float32)

    xT = {}
    for b in range(B):
        xT[("q", b)] = np.ascontiguousarray(q[b].T.astype(BF))
        xT[("k", b)] = np.ascontiguousarray(k[b].T.astype(BF))
        xT[("v", b)] = np.ascontiguousarray(v[b].T.astype(BF))
    in_maps = []
    for g in range(8):
        b, hg = g // 2, g % 2
        sl = slice(hg * 512, (hg + 1) * 512)
        def shuf(w):  # [1024, n] -> [128, 8*n]: row c*128+p -> [p, c, :]
            n = w.shape[1]
            return np.ascontiguousarray(
                w.reshape(CC, 128, n).transpose(1, 0, 2).reshape(128, CC * n)
                .astype(BF))

        def shuf_pm(w):  # [1024, 512] -> [128, (pair, c, 128)] pair-major
            return np.ascontiguousarray(
                w.reshape(CC, 128, PAIRS, 128).transpose(1, 2, 0, 3)
                .reshape(128, PAIRS * CC * 128).astype(BF))

        wo_sl = w_o[sl, :]  # [512, 1024]: row i*128+p -> [p, i, :]
        wo_sh = np.ascontiguousarray(
            wo_sl.reshape(PAIRS, 128, D).transpose(1, 0, 2)
            .reshape(128, PAIRS * D).astype(BF))
        in_maps.append({
            "xq": xT[("q", b)], "xk": xT[("k", b)], "xv": xT[("v", b)],
            "wq": shuf_pm(w_q[:, sl]),
            "wk": shuf_pm(w_k[:, sl]),
            "wv": shuf(w_v[:, sl]),
            "wo": wo_sh,
            "bq": np.ascontiguousarray(b_q[sl].reshape(PAIRS, 128)),
            "ones": np.ones((128, HPC), dtype=BF),
        })

    _CACHE["in_maps"] = in_maps
    res = run_bass_kernel_spmd(nc, in_maps, list(range(8)), trace=False)
    outs = [r["yt"] for r in res.results]

    corr = b_v @ w_o + b_o  # [1024]
    y = np.empty((B, S, D), dtype=np.float32)
    for b in range(B):
        y[b] = outs[2 * b].T + outs[2 * b + 1].T + corr
    return y
